# revision 34
# baseline (speedup 1.0000x reference)
"""MoE down-projection (grouped GEMM + topk combine) on 8 Trainium2 cores.

Strategy: expert-parallel. Each of the 8 cores owns E/8 = 16 experts and
receives (a) its experts' weight slabs and (b) the x rows routed to those
experts, gathered+gate-scaled+transposed on host, padded per expert to a
fixed capacity C. The device kernel is a block-diagonal grouped GEMM.
Weights stream through the PE as the moving operand (full rate); the few
x rows per expert are the stationary operand. G = 128//C h-chunks of one
expert run concurrently in separate PE column groups (tile_position),
each owning a contiguous C-partition range of a [128, H/G] PSUM tile.
The psum->sbuf copies then COMPACT the G groups' valid rows into the
free dim, producing a row-major [n, H] tile per expert so a single small
store moves only the real token rows. Host scatter-adds the rows back
into the [T, H] output.

The kernel is HBM-bandwidth bound on the weight stream (16 MiB/core fp8
at the ~430 GB/s = 16 engines x 27 GB/s per-core DMA ceiling), so the
default config stores w as fp8 E3M4 (per-expert scale folded into the x
rows) and the y output as compacted bf16 rows — total ~18.9 MiB/core
moved vs 512 MiB f32 for the naive form, at ~1.3e-2 relative error
(gate is 2e-2). Experts are assigned to (core, slot) in count-sorted
rank groups so one per-slot row count is tight for the whole SPMD
program and per-core load balances.

Hardware behaviors this kernel is shaped around (all measured here):
- every HWDGE DMA trigger costs ~0.6-0.9us of issuing-engine queue
  time; gpsimd SWDGE costs ~2.2us of ucode per store — so DMA COUNT is
  a first-class budget (w: 11 tapered slab pieces; y: 16 pair stores).
- the sync+scalar HWDGE queues share 8 HW completion-sem lanes; DMAs
  past 8-in-flight wait for lane recycling AT TRIGGER TIME.
- a DMA's descriptor lines fan over the 16 data engines restarting at
  the first engine each trigger: line counts that are multiples of 16
  (rows padded to 16/32) keep per-engine bytes exactly even; ragged
  stores piled 5.5x average on E64 and stretched the whole stream.
- any object (psum tile, sbuf tile, store) consumed by BOTH copy
  engines makes the tile framework fold multi-waits into a transitive
  vector->scalar wait chain (1-wait-per-instruction HW rule) that
  serializes all psum->sbuf copies; the vector and scalar pipelines
  here are fully object-disjoint (own psum tile, own ot tile, own
  store) and the host reorders the h-chunk halves.
- weight-release granularity tapers (2-expert pieces, then per-expert,
  then half-expert) so the copy engines (~1.4us/expert each) never
  inherit a multi-expert burst at the contended stream end.

Hardcoded problem shape (from the problem spec):
  x: [2048, 512] f32, w: [128, 512, 2048] f32,
  chosen_experts: [1024, 2] int, expert_weight: [1024, 2] f32 -> out [1024, 2048] f32
"""

import numpy as np

T = 1024
K_TOP = 2
E = 128
I_DIM = 512
H = 2048
N_CORES = 8
EPC = E // N_CORES  # experts per core = 16
P = 128             # partitions
I_CHUNKS = I_DIM // P       # 4
H_CHUNK = 512               # matmul moving free dim (fp32 PSUM bank)
H_CHUNKS = H // H_CHUNK     # 4

# matmul dtype config: name -> (w dtype, x dtype, y dtype)
#   float8e3  : w E3M4 (per-expert scaled), x bf16, y bf16 — half DMA traffic
#   float8e3x : both operands E3M4 (if mixed-dtype matmul is unsupported)
#   bfloat16  : both bf16, y f32
#   float32 / float32r: exact / reduced-precision f32
DT_CONFIGS = {
    "float8e3": ("float8e3", "bfloat16", "bfloat16"),
    "float8e3x": ("float8e3", "float8e3", "bfloat16"),
    "bfloat16": ("bfloat16", "bfloat16", "float32"),
    "float32": ("float32", "float32", "float32"),
    "float32r": ("float32r", "float32r", "float32"),
}
DEFAULT_DTYPE = "float8e3"
E3M4_SCALE_TARGET = 14.0  # keep clear of the 15.5 e3m4 max normal

_cache = {}


def _w_bytes(w_dtn):
    return 1 if w_dtn == "float8e3" else (2 if w_dtn == "bfloat16" else 4)


def _eps(w_dtn):
    """Experts per weight-slab BUFFER (4 MiB fp8 -> 4 slabs resident =
    the whole 16 MiB working set). DMA granularity within a slab is
    finer (see issue_slab): the piece count is kept near the 8 HW
    completion-sem lanes shared by the sync+scalar DGE queues — DMAs
    past 8-in-flight wait for lane recycling at trigger time, which can
    serialize the queue behind data completions (measured +6us when
    splits pushed the count to 15)."""
    mib = 4 if _w_bytes(w_dtn) == 1 else 2
    return max(1, mib * 1024 * 1024 // (P * I_CHUNKS * H * _w_bytes(w_dtn)))


def _build(C: int, dt_name: str, ns: tuple | None = None):
    """ns: per-slot valid row counts (same for every core by construction —
    the host assigns experts to slots in count-sorted rank groups). When
    given, y stores move only those rows."""
    import concourse.mybir as mybir
    import concourse.tile as tile
    from concourse import bacc

    w_dtn, x_dtn, y_dtn = DT_CONFIGS[dt_name]
    w_dt = getattr(mybir.dt, w_dtn)
    x_dt = getattr(mybir.dt, x_dtn)
    y_dt = getattr(mybir.dt, y_dtn)
    w_bytes = _w_bytes(w_dtn)
    EPS = _eps(w_dtn)
    SLABS = EPC // EPS
    SLAB_COLS = EPS * I_CHUNKS * H
    # G = 128//C PE column groups run one expert's G h-chunks concurrently;
    # expert b's H chunk h goes to psum partitions (h%G)*C..+C, bank cols
    # (h//G)*512..+512, so casts and stores use all 128 partitions. Host
    # unpacks. fp32 rejects tile_position col-tiling.
    G = max(1, P // C)
    if H_CHUNKS % G != 0 or w_dtn not in ("bfloat16", "float8e3"):
        G = 1
    NB = H_CHUNKS // G
    PPART = G * C
    # keep the whole weight working set resident when it fits (fp8: 16 MiB)
    wbufs = SLABS if w_bytes == 1 else (6 if w_bytes == 2 else 3)
    # ot tiles are small ([C, H/2] y_dt); deep rotation keeps the
    # cast->store WAR chain from ever pacing the compute pipeline
    obufs = 8 if w_bytes <= 2 else 2
    # two psum tiles per expert (one per copy engine) x 4 in flight = all
    # 8 banks; G == 1 falls back to one tile per expert
    pbufs = 4 if G > 1 else 2

    nc = bacc.Bacc()
    # wc host-prearranged: [k, p, e*ICH*H + i*H + h] = w[k*EPS+e, i*128+p, h]
    # so each partition's slab line is 1 contiguous run per DMA
    wc = nc.declare_dram_parameter("wc", [SLABS, P, SLAB_COLS], w_dt, isOutput=False)
    # x host-prearranged: [p, i*EC + c] = x[i*128+p, c] (EC = EPC*C) so the
    # whole stationary operand arrives in ONE small DMA before the w flood
    xT = nc.declare_dram_parameter("xT", [P, I_CHUNKS * EPC * C], x_dt, isOutput=False)
    # y rows are stored compacted: y[b, r] = full H row for valid row r < ns[b]
    y = nc.declare_dram_parameter("y", [EPC, C, H], y_dt, isOutput=True)

    with tile.TileContext(nc) as tc:
        with (
            tc.tile_pool(name="wp", bufs=wbufs) as wp,
            tc.tile_pool(name="xp", bufs=1) as xp,
            tc.tile_pool(name="ppv", bufs=pbufs, space="PSUM") as ppv,
            tc.tile_pool(name="pps", bufs=pbufs, space="PSUM") as pps,
            tc.tile_pool(name="opv", bufs=obufs) as opv,
            tc.tile_pool(name="ops", bufs=obufs) as ops,
        ):
            # x rows (stationary operands) go out on the scalar HWDGE queue:
            # the sync ring then issues w slab triggers back-to-back from the
            # first kernel instruction, starting the weight stream ~0.9us
            # earlier. x interleaves with slab 0 on the shared engines and
            # still lands long before the first matmul needs it. (x is not
            # compute-gated, so it cannot block anything through scalar's
            # sem lanes the way compute-gated y stores would.)
            EC = EPC * C
            xt_all = xp.tile([P, I_CHUNKS * EC], x_dt, tag="x", name="x")
            nc.scalar.dma_start(out=xt_all[:], in_=xT[:])
            xtiles = [xt_all[:, i * EC:(i + 1) * EC] for i in range(I_CHUNKS)]

            def issue_slab(k):
                # Weight-release granularity tapers toward the stream end:
                # early slabs whole (maximum trigger slack — the first 8
                # HWDGE DMAs hold the 8 completion-sem lanes and issue
                # upfront), then 2-expert halves, then per-expert, and the
                # very last expert in two i-chunk halves. Coarse releases
                # bunch experts into the copy engines (1.38us/expert drain
                # vs 2.66us/expert stream pace is fine steady-state, but a
                # 4-expert burst at the contended stream end added ~5us of
                # tail); the taper keeps the tail per-expert while w DMA
                # count stays at 10 (+x = 11, one benign recycle-wait).
                wt = wp.tile([P, SLAB_COLS], w_dt, tag="w0",
                             name=f"w{k}", bufs=wbufs)
                ecols = I_CHUNKS * H
                if k == SLABS - 1 and EPS > 1:
                    for e in range(EPS - 1):
                        nc.sync.dma_start(out=wt[:, e * ecols:(e + 1) * ecols],
                                          in_=wc[k, :, e * ecols:(e + 1) * ecols])
                    lo = (EPS - 1) * ecols
                    half = I_CHUNKS // 2 * H
                    for h2 in range(2):
                        nc.sync.dma_start(
                            out=wt[:, lo + h2 * half:lo + (h2 + 1) * half],
                            in_=wc[k, :, lo + h2 * half:lo + (h2 + 1) * half])
                elif EPS >= 4:
                    # 2-expert release pieces: a whole 4-expert slab dumps
                    # ~5.5us of copy work on each copy engine at once while
                    # the stream feeds ~1us/expert of slack — the backlog
                    # compounds under contention and drains as tail
                    hcols = (EPS // 2) * ecols
                    for h2 in range(2):
                        nc.sync.dma_start(
                            out=wt[:, h2 * hcols:(h2 + 1) * hcols],
                            in_=wc[k, :, h2 * hcols:(h2 + 1) * hcols])
                else:
                    nc.sync.dma_start(out=wt[:], in_=wc[k])
                return wt

            # issue every slab DMA upfront when all buffers are resident
            # (fp8: 8 x 2 MiB); otherwise stream with buffer rotation
            wts = {k: issue_slab(k) for k in range(min(wbufs, SLABS))}

            # Copy-engine split: PE column groups g < GV accumulate in the
            # "vector" psum tile (partitions [0, GV*C)), groups g >= GV in
            # the "scalar" psum tile (partitions [GV*C, PPART)). Vector
            # copies compact the first half's h-chunks into otv, scalar the
            # second half's into ots, and each half has its OWN store.
            # CRITICAL: the two engines get fully DISJOINT object graphs —
            # separate psum tiles, separate ot tiles, separate stores — so
            # NO instruction ever depends on both engines' progress. Any
            # shared object (one psum tile, one ot tile, or one combined
            # store) makes the tile framework fold the resulting multi-wait
            # into a transitive V->S->V->S chain (single-wait-per-
            # instruction HW rule), serializing ALL copies at 2.76us/expert
            # — slower than the 2.66us/expert weight stream (measured
            # repeatedly as an ~10us tail). Disjoint halves run at
            # 1.38us/expert per engine. Host unpacks the h-chunk order
            # (vector's h-chunks first, then scalar's).
            GV = max(1, G // 2)  # column groups handled by vector
            NV = GV * NB if G > 1 else G * NB
            vsplit = NV * H_CHUNK
            ssplit = H - vsplit

            for b in range(EPC):
                k = b // EPS
                if k not in wts:
                    wts[k] = issue_slab(k)
                wt = wts[k]
                wo = (b % EPS) * I_CHUNKS * H
                psv = ppv.tile([PPART, NB * H_CHUNK], mybir.dt.float32,
                               tag="psv", name=f"psv{b}")
                pss = (pps.tile([PPART, NB * H_CHUNK], mybir.dt.float32,
                                tag="pss", name=f"pss{b}") if G > 1 else None)
                for i in range(I_CHUNKS):
                    for h in range(H_CHUNKS):
                        g, bank = h % G, h // G
                        ps = psv if (G == 1 or g < GV) else pss
                        nc.tensor.matmul(
                            ps[g * C:(g + 1) * C,
                               bank * H_CHUNK:(bank + 1) * H_CHUNK],
                            lhsT=xtiles[i][:, b * C:(b + 1) * C],
                            rhs=wt[:, wo + i * H + h * H_CHUNK:
                                   wo + i * H + (h + 1) * H_CHUNK],
                            start=(i == 0),
                            stop=(i == I_CHUNKS - 1),
                            tile_position=(0, g * C) if G > 1 else None,
                        )
                # Compact each half's valid rows into the FREE dim during
                # the psum->sbuf cast: (g, bank) block -> [n, 512] col
                # block, giving row-major [n, H/2] tiles. Partition bases
                # stay 32-aligned (engine ops reject unaligned bases) and
                # y stores then move only real token rows — y bytes drop
                # ~2x off the shared DMA engines that carry the w stream.
                #
                # Stores cover an EXPERT PAIR (rows padded to the pair max;
                # slots are count-sorted so the overhead is ~6%): 16 HWDGE
                # DMAs instead of 32. Every HWDGE DMA costs ~0.7us of queue
                # time, recycles one of only 8 HW completion-sem lanes
                # (a 33rd+ DMA serializes the tail at ~0.94us/store,
                # measured as a 10us dribble), and small-line DMAs fan
                # poorly over the 16 data engines (lines restart at E64
                # each trigger).
                n = C if ns is None else max(1, min(int(ns[b]), C))
                if b % 2 == 0:
                    otv = opv.tile([C, 2 * vsplit], y_dt, tag="ov",
                                   name=f"ov{b}")
                    ots = (ops.tile([C, 2 * ssplit], y_dt, tag="os",
                                    name=f"os{b}") if ssplit else None)
                    pair_n = n
                vo = (b % 2) * vsplit
                so = (b % 2) * ssplit
                for g in range(G):
                    for bank in range(NB):
                        if G == 1 or g < GV:
                            j = g * NB + bank
                            nc.vector.tensor_copy(
                                out=otv[0:n, vo + j * H_CHUNK:
                                        vo + (j + 1) * H_CHUNK],
                                in_=psv[g * C:g * C + n,
                                        bank * H_CHUNK:(bank + 1) * H_CHUNK])
                        else:
                            j = (g - GV) * NB + bank
                            nc.scalar.copy(
                                out=ots[0:n, so + j * H_CHUNK:
                                        so + (j + 1) * H_CHUNK],
                                in_=pss[g * C:g * C + n,
                                        bank * H_CHUNK:(bank + 1) * H_CHUNK])
                if b % 2 == 1 or b == EPC - 1:
                    # ALL stores ride the sync HWDGE queue: it is idle after
                    # the upfront slab triggers (wbufs == SLABS on the fp8
                    # path), so a compute-gated store at its head blocks
                    # nothing. A DMA trigger costs ~0.8us of ENGINE time on
                    # whichever engine issues it — on scalar that stole
                    # copy throughput (measured +5us tail); sync has
                    # nothing else to do. gpsimd SWDGE is NOT usable: its
                    # ucode spends ~2.2us per store generating small
                    # packets (measured +15us).
                    p0 = b - (b % 2)
                    ne = b - p0 + 1
                    # rows padded to a multiple of 16: HWDGE fans a DMA's
                    # descriptor lines over the 16 data engines starting at
                    # the first engine each trigger, so off-multiple line
                    # counts pile the remainder onto E64/E65 (measured:
                    # E64 carried 5.5x the average y bytes and became the
                    # critical engine, +13us). 16/32-row stores wrap all
                    # 16 engines exactly; the padding costs ~0.4 MiB.
                    rows = min(C, -(-max(pair_n, n) // 16) * 16)
                    yv = y[p0:p0 + ne, 0:rows, 0:vsplit]
                    nc.sync.dma_start(
                        out=yv.rearrange("e r h -> r e h"),
                        in_=otv[0:rows, 0:ne * vsplit])
                    if ots is not None:
                        ys = y[p0:p0 + ne, 0:rows, vsplit:H]
                        nc.sync.dma_start(
                            out=ys.rearrange("e r h -> r e h"),
                            in_=ots[0:rows, 0:ne * ssplit])
    nc.compile()
    return nc


def _get_nc(C: int, dt_name: str, ns: tuple | None = None):
    key = (C, dt_name, ns)
    if key not in _cache:
        _cache[key] = _build(C, dt_name, ns)
    return _cache[key]


def _np_dt(name):
    import ml_dtypes
    return {
        "float8e3": ml_dtypes.float8_e3m4,
        "bfloat16": ml_dtypes.bfloat16,
        "float32": np.float32,
        "float32r": np.float32,
    }[name]


def _prepare(x, w, chosen_experts, expert_weight, dt_name):
    """Host-side routing. Returns (C, ns, in_maps, row_lists) where
    row_lists[c][s] is the array of global row ids for core c, expert slot
    s, and ns[s] the per-slot valid row count baked into the kernel."""
    w_dtn, x_dtn, _ = DT_CONFIGS[dt_name]
    x = np.asarray(x, dtype=np.float32)
    w = np.asarray(w, dtype=np.float32)
    ce = np.asarray(chosen_experts).astype(np.int64).reshape(-1)      # [T*K]
    gw = np.asarray(expert_weight, dtype=np.float32).reshape(-1)      # [T*K]

    counts = np.bincount(ce, minlength=E)
    C = max(32, int(np.ceil(counts.max() / 32.0) * 32))

    order = np.argsort(ce, kind="stable")
    starts = np.zeros(E + 1, dtype=np.int64)
    np.cumsum(counts, out=starts[1:])

    xs = x * gw[:, None]  # fold router gate into rows (fp32)

    if w_dtn == "float8e3":
        # per-expert scale into the e3m4 range; inverse folded into x rows
        s = E3M4_SCALE_TARGET / np.maximum(
            np.abs(w).max(axis=(1, 2)), 1e-30)                        # [E]
    else:
        s = np.ones(E, dtype=np.float32)

    # Assign experts to (core, slot) in count-sorted rank groups: slot b on
    # every core gets an expert of rank group b, so one per-slot row count
    # (the group max) is tight for the whole SPMD program, y stores move
    # only real rows, and per-core load balances.
    rank = np.argsort(-counts, kind="stable")          # expert ids, big first
    assign = rank.reshape(EPC, N_CORES)                # [slot, core]
    ns = tuple(int(counts[assign[b]].max()) for b in range(EPC))

    EPS = _eps(w_dtn)
    in_maps, row_lists = [], []
    for c in range(N_CORES):
        xg = np.zeros((EPC * C, I_DIM), dtype=np.float32)
        rows_c = []
        for sl in range(EPC):
            e = int(assign[sl, c])
            rows = order[starts[e]:starts[e + 1]]
            xg[sl * C: sl * C + len(rows)] = xs[rows] * (1.0 / s[e])
            rows_c.append(rows)
        # [b, i*128+p, h] -> [k, p, e*ICH*H + i*H + h] (b = k*EPS+e):
        # contiguous per-partition slab lines, EPS experts per DMA slab
        eids = assign[:, c]
        wcore = (
            (w[eids] * s[eids, None, None])
            .reshape(EPC // EPS, EPS, I_CHUNKS, P, H)
            .transpose(0, 3, 1, 2, 4)
            .reshape(EPC // EPS, P, EPS * I_CHUNKS * H)
        )
        # [c, i*128+p] -> [p, i*EC + c]: one resident stationary tile
        xre = (
            xg.reshape(EPC * C, I_CHUNKS, P)
            .transpose(2, 1, 0)
            .reshape(P, I_CHUNKS * EPC * C)
        )
        in_maps.append({
            "wc": np.ascontiguousarray(wcore).astype(_np_dt(w_dtn)),
            "xT": np.ascontiguousarray(xre).astype(_np_dt(x_dtn)),
        })
        row_lists.append(rows_c)
    return C, ns, in_maps, row_lists


def _combine(results, row_lists, C, dt_name):
    # device stores row-major [n, H] per expert slot (compacted valid rows)
    # with H blocks permuted: vector-copied h-chunks first, then scalar's
    G = max(1, P // C)
    if H_CHUNKS % G != 0 or DT_CONFIGS[dt_name][0] not in ("bfloat16", "float8e3"):
        G = 1
    NB = H_CHUNKS // G
    NCOP = G * NB
    GV = max(1, G // 2)
    NV = GV * NB if G > 1 else G * NB
    blocks = [0] * NCOP  # blocks[j] = h-chunk stored in device col block j
    for g in range(G):
        for bank in range(NB):
            if G == 1 or g < GV:
                j = g * NB + bank
            else:
                j = NV + (g - GV) * NB + bank
            blocks[j] = bank * G + g
    yfull = np.empty((T * K_TOP, H), dtype=np.float32)
    for c in range(N_CORES):
        yc = np.asarray(results[c]["y"], dtype=np.float32)  # [EPC, C, H]
        yb = yc.reshape(EPC, C, NCOP, H_CHUNK)
        nat = np.empty_like(yb)
        nat[:, :, blocks, :] = yb
        nat = nat.reshape(EPC, C, H)
        for s, rows in enumerate(row_lists[c]):
            if len(rows):
                yfull[rows] = nat[s, : len(rows)]
    return yfull[0::2] + yfull[1::2]


def run(x, w, chosen_experts, expert_weight, dt_name=DEFAULT_DTYPE, **spmd_kwargs):
    from concourse.bass_utils import run_bass_kernel_spmd

    C, ns, in_maps, row_lists = _prepare(x, w, chosen_experts, expert_weight, dt_name)
    nc = _get_nc(C, dt_name, ns)
    res = run_bass_kernel_spmd(nc, in_maps, core_ids=list(range(N_CORES)), **spmd_kwargs)
    out = _combine(res.results, row_lists, C, dt_name)
    return out, res


def kernel(x, w, chosen_experts, expert_weight):
    out, _ = run(x, w, chosen_experts, expert_weight)
    return out



# revision 35
# speedup vs baseline: 1.1193x; 1.1193x over previous
"""MoE down-projection (grouped GEMM + topk combine) on 8 Trainium2 cores.

Strategy: expert-parallel. Each of the 8 cores owns E/8 = 16 experts and
receives (a) its experts' weight slabs and (b) the x rows routed to those
experts, gathered+gate-scaled+transposed on host, padded per expert to a
fixed capacity C. The device kernel is a block-diagonal grouped GEMM.
Weights stream through the PE as the moving operand (full rate); the few
x rows per expert are the stationary operand. G = 128//C h-chunks of one
expert run concurrently in separate PE column groups (tile_position),
each owning a contiguous C-partition range of a [128, H/G] PSUM tile.
The psum->sbuf copies then COMPACT the G groups' valid rows into the
free dim, producing a row-major [n, H] tile per expert so a single small
store moves only the real token rows. Host scatter-adds the rows back
into the [T, H] output.

The kernel is HBM-bandwidth bound on the weight stream (16 MiB/core fp8
at the ~430 GB/s = 16 engines x 27 GB/s per-core DMA ceiling), so the
default config stores w as fp8 E3M4 (per-expert scale folded into the x
rows) and the y output as compacted bf16 rows — total ~18.9 MiB/core
moved vs 512 MiB f32 for the naive form, at ~1.3e-2 relative error
(gate is 2e-2). Experts are assigned to (core, slot) in count-sorted
rank groups so one per-slot row count is tight for the whole SPMD
program and per-core load balances.

Hardware behaviors this kernel is shaped around (all measured here):
- every HWDGE DMA trigger costs ~0.6-0.9us of issuing-engine queue
  time; gpsimd SWDGE costs ~2.2us of ucode per store — so DMA COUNT is
  a first-class budget (w: 11 tapered slab pieces; y: 16 pair stores).
- the sync+scalar HWDGE queues share 8 HW completion-sem lanes; DMAs
  past 8-in-flight wait for lane recycling AT TRIGGER TIME.
- a DMA's descriptor lines fan over the 16 data engines restarting at
  the first engine each trigger: line counts that are multiples of 16
  (rows padded to 16/32) keep per-engine bytes exactly even; ragged
  stores piled 5.5x average on E64 and stretched the whole stream.
- any object (psum tile, sbuf tile, store) consumed by BOTH copy
  engines makes the tile framework fold multi-waits into a transitive
  vector->scalar wait chain (1-wait-per-instruction HW rule) that
  serializes all psum->sbuf copies; the vector and scalar pipelines
  here are fully object-disjoint (own psum tile, own ot tile, own
  store) and the host reorders the h-chunk halves.
- weight-release granularity tapers (2-expert pieces, then per-expert,
  then half-expert) so the copy engines (~1.4us/expert each) never
  inherit a multi-expert burst at the contended stream end.

Hardcoded problem shape (from the problem spec):
  x: [2048, 512] f32, w: [128, 512, 2048] f32,
  chosen_experts: [1024, 2] int, expert_weight: [1024, 2] f32 -> out [1024, 2048] f32
"""

import numpy as np

T = 1024
K_TOP = 2
E = 128
I_DIM = 512
H = 2048
N_CORES = 8
EPC = E // N_CORES  # experts per core = 16
P = 128             # partitions
I_CHUNKS = I_DIM // P       # 4
H_CHUNK = 512               # matmul moving free dim (fp32 PSUM bank)
H_CHUNKS = H // H_CHUNK     # 4

# matmul dtype config: name -> (w dtype, x dtype, y dtype)
#   float8e3  : w E3M4 (per-expert scaled), x bf16, y bf16 — half DMA traffic
#   float8e3x : both operands E3M4 (if mixed-dtype matmul is unsupported)
#   bfloat16  : both bf16, y f32
#   float32 / float32r: exact / reduced-precision f32
DT_CONFIGS = {
    "float8e3": ("float8e3", "bfloat16", "bfloat16"),
    "float8e3x": ("float8e3", "float8e3", "bfloat16"),
    "bfloat16": ("bfloat16", "bfloat16", "float32"),
    "float32": ("float32", "float32", "float32"),
    "float32r": ("float32r", "float32r", "float32"),
}
DEFAULT_DTYPE = "float8e3"
E3M4_SCALE_TARGET = 14.0  # keep clear of the 15.5 e3m4 max normal

_cache = {}


def _w_bytes(w_dtn):
    return 1 if w_dtn == "float8e3" else (2 if w_dtn == "bfloat16" else 4)


def _eps(w_dtn):
    """Experts per weight-slab BUFFER (4 MiB fp8 -> 4 slabs resident =
    the whole 16 MiB working set). DMA granularity within a slab is
    finer (see issue_slab): the piece count is kept near the 8 HW
    completion-sem lanes shared by the sync+scalar DGE queues — DMAs
    past 8-in-flight wait for lane recycling at trigger time, which can
    serialize the queue behind data completions (measured +6us when
    splits pushed the count to 15)."""
    mib = 4 if _w_bytes(w_dtn) == 1 else 2
    return max(1, mib * 1024 * 1024 // (P * I_CHUNKS * H * _w_bytes(w_dtn)))


def _build(C: int, dt_name: str, ns: tuple | None = None):
    """ns: per-slot valid row counts (same for every core by construction —
    the host assigns experts to slots in count-sorted rank groups). When
    given, y stores move only those rows."""
    import concourse.mybir as mybir
    import concourse.tile as tile
    from concourse import bacc

    w_dtn, x_dtn, y_dtn = DT_CONFIGS[dt_name]
    w_dt = getattr(mybir.dt, w_dtn)
    x_dt = getattr(mybir.dt, x_dtn)
    y_dt = getattr(mybir.dt, y_dtn)
    w_bytes = _w_bytes(w_dtn)
    EPS = _eps(w_dtn)
    SLABS = EPC // EPS
    SLAB_COLS = EPS * I_CHUNKS * H
    # G = 128//C PE column groups run one expert's G h-chunks concurrently;
    # expert b's H chunk h goes to psum partitions (h%G)*C..+C, bank cols
    # (h//G)*512..+512, so casts and stores use all 128 partitions. Host
    # unpacks. fp32 rejects tile_position col-tiling.
    G = max(1, P // C)
    if H_CHUNKS % G != 0 or w_dtn not in ("bfloat16", "float8e3"):
        G = 1
    NB = H_CHUNKS // G
    PPART = G * C
    # keep the whole weight working set resident when it fits (fp8: 16 MiB)
    wbufs = SLABS if w_bytes == 1 else (6 if w_bytes == 2 else 3)
    # ot tiles are small ([C, H/2] y_dt); deep rotation keeps the
    # cast->store WAR chain from ever pacing the compute pipeline
    obufs = 8 if w_bytes <= 2 else 2
    # two psum tiles per expert (one per copy engine), each NB banks deep;
    # size the rotation to the 8 PSUM banks (G=4: 2x4x1, G=2: 2x2x2).
    # G == 1 uses one tile of NB banks per expert.
    pbufs = max(1, 4 // NB) if G > 1 else max(1, 8 // (2 * NB))

    nc = bacc.Bacc()
    # wc host-prearranged: [k, p, e*ICH*H + i*H + h] = w[k*EPS+e, i*128+p, h]
    # so each partition's slab line is 1 contiguous run per DMA
    wc = nc.declare_dram_parameter("wc", [SLABS, P, SLAB_COLS], w_dt, isOutput=False)
    # x host-prearranged: [p, i*EC + c] = x[i*128+p, c] (EC = EPC*C) so the
    # whole stationary operand arrives in ONE small DMA before the w flood
    xT = nc.declare_dram_parameter("xT", [P, I_CHUNKS * EPC * C], x_dt, isOutput=False)
    # y rows are stored compacted: y[b, r] = full H row for valid row r < ns[b]
    y = nc.declare_dram_parameter("y", [EPC, C, H], y_dt, isOutput=True)

    with tile.TileContext(nc) as tc:
        with (
            tc.tile_pool(name="wp", bufs=wbufs) as wp,
            tc.tile_pool(name="xp", bufs=1) as xp,
            tc.tile_pool(name="ppv", bufs=pbufs, space="PSUM") as ppv,
            tc.tile_pool(name="pps", bufs=pbufs, space="PSUM") as pps,
            tc.tile_pool(name="opv", bufs=obufs) as opv,
            tc.tile_pool(name="ops", bufs=obufs) as ops,
        ):
            # x rows (stationary operands) go out on the scalar HWDGE queue:
            # the sync ring then issues w slab triggers back-to-back from the
            # first kernel instruction, starting the weight stream ~0.9us
            # earlier. x interleaves with slab 0 on the shared engines and
            # still lands long before the first matmul needs it. (x is not
            # compute-gated, so it cannot block anything through scalar's
            # sem lanes the way compute-gated y stores would.)
            EC = EPC * C
            xt_all = xp.tile([P, I_CHUNKS * EC], x_dt, tag="x", name="x")
            nc.scalar.dma_start(out=xt_all[:], in_=xT[:])
            xtiles = [xt_all[:, i * EC:(i + 1) * EC] for i in range(I_CHUNKS)]

            def issue_slab(k):
                # Weight-release granularity tapers toward the stream end:
                # early slabs whole (maximum trigger slack — the first 8
                # HWDGE DMAs hold the 8 completion-sem lanes and issue
                # upfront), then 2-expert halves, then per-expert, and the
                # very last expert in two i-chunk halves. Coarse releases
                # bunch experts into the copy engines (1.38us/expert drain
                # vs 2.66us/expert stream pace is fine steady-state, but a
                # 4-expert burst at the contended stream end added ~5us of
                # tail); the taper keeps the tail per-expert while w DMA
                # count stays at 10 (+x = 11, one benign recycle-wait).
                wt = wp.tile([P, SLAB_COLS], w_dt, tag="w0",
                             name=f"w{k}", bufs=wbufs)
                ecols = I_CHUNKS * H
                if k == SLABS - 1 and EPS > 1:
                    for e in range(EPS - 1):
                        nc.sync.dma_start(out=wt[:, e * ecols:(e + 1) * ecols],
                                          in_=wc[k, :, e * ecols:(e + 1) * ecols])
                    lo = (EPS - 1) * ecols
                    half = I_CHUNKS // 2 * H
                    for h2 in range(2):
                        nc.sync.dma_start(
                            out=wt[:, lo + h2 * half:lo + (h2 + 1) * half],
                            in_=wc[k, :, lo + h2 * half:lo + (h2 + 1) * half])
                elif EPS >= 4:
                    # 2-expert release pieces: a whole 4-expert slab dumps
                    # ~5.5us of copy work on each copy engine at once while
                    # the stream feeds ~1us/expert of slack — the backlog
                    # compounds under contention and drains as tail
                    hcols = (EPS // 2) * ecols
                    for h2 in range(2):
                        nc.sync.dma_start(
                            out=wt[:, h2 * hcols:(h2 + 1) * hcols],
                            in_=wc[k, :, h2 * hcols:(h2 + 1) * hcols])
                else:
                    nc.sync.dma_start(out=wt[:], in_=wc[k])
                return wt

            # issue every slab DMA upfront when all buffers are resident
            # (fp8: 8 x 2 MiB); otherwise stream with buffer rotation
            wts = {k: issue_slab(k) for k in range(min(wbufs, SLABS))}

            # Copy-engine split: PE column groups g < GV accumulate in the
            # "vector" psum tile (partitions [0, GV*C)), groups g >= GV in
            # the "scalar" psum tile (partitions [GV*C, PPART)). Vector
            # copies compact the first half's h-chunks into otv, scalar the
            # second half's into ots, and each half has its OWN store.
            # CRITICAL: the two engines get fully DISJOINT object graphs —
            # separate psum tiles, separate ot tiles, separate stores — so
            # NO instruction ever depends on both engines' progress. Any
            # shared object (one psum tile, one ot tile, or one combined
            # store) makes the tile framework fold the resulting multi-wait
            # into a transitive V->S->V->S chain (single-wait-per-
            # instruction HW rule), serializing ALL copies at 2.76us/expert
            # — slower than the 2.66us/expert weight stream (measured
            # repeatedly as an ~10us tail). Disjoint halves run at
            # 1.38us/expert per engine. Host unpacks the h-chunk order
            # (vector's h-chunks first, then scalar's).
            GV = max(1, G // 2)  # column groups handled by vector
            NV = GV * NB if G > 1 else G * NB
            vsplit = NV * H_CHUNK
            ssplit = H - vsplit

            for b in range(EPC):
                k = b // EPS
                if k not in wts:
                    wts[k] = issue_slab(k)
                wt = wts[k]
                wo = (b % EPS) * I_CHUNKS * H
                psv = ppv.tile([PPART, NB * H_CHUNK], mybir.dt.float32,
                               tag="psv", name=f"psv{b}")
                pss = (pps.tile([PPART, NB * H_CHUNK], mybir.dt.float32,
                                tag="pss", name=f"pss{b}") if G > 1 else None)
                for i in range(I_CHUNKS):
                    for h in range(H_CHUNKS):
                        g, bank = h % G, h // G
                        ps = psv if (G == 1 or g < GV) else pss
                        nc.tensor.matmul(
                            ps[g * C:(g + 1) * C,
                               bank * H_CHUNK:(bank + 1) * H_CHUNK],
                            lhsT=xtiles[i][:, b * C:(b + 1) * C],
                            rhs=wt[:, wo + i * H + h * H_CHUNK:
                                   wo + i * H + (h + 1) * H_CHUNK],
                            start=(i == 0),
                            stop=(i == I_CHUNKS - 1),
                            tile_position=(0, g * C) if G > 1 else None,
                        )
                # Compact each half's valid rows into the FREE dim during
                # the psum->sbuf cast: (g, bank) block -> [n, 512] col
                # block, giving row-major [n, H/2] tiles. Partition bases
                # stay 32-aligned (engine ops reject unaligned bases) and
                # y stores then move only real token rows — y bytes drop
                # ~2x off the shared DMA engines that carry the w stream.
                #
                # Stores cover an EXPERT PAIR (rows padded to the pair max;
                # slots are count-sorted so the overhead is ~6%): 16 HWDGE
                # DMAs instead of 32. Every HWDGE DMA costs ~0.7us of queue
                # time, recycles one of only 8 HW completion-sem lanes
                # (a 33rd+ DMA serializes the tail at ~0.94us/store,
                # measured as a 10us dribble), and small-line DMAs fan
                # poorly over the 16 data engines (lines restart at E64
                # each trigger).
                n = C if ns is None else max(1, min(int(ns[b]), C))
                if b % 2 == 0:
                    otv = opv.tile([C, 2 * vsplit], y_dt, tag="ov",
                                   name=f"ov{b}")
                    ots = (ops.tile([C, 2 * ssplit], y_dt, tag="os",
                                    name=f"os{b}") if ssplit else None)
                    pair_n = n
                vo = (b % 2) * vsplit
                so = (b % 2) * ssplit
                for g in range(G):
                    for bank in range(NB):
                        if G == 1 or g < GV:
                            j = g * NB + bank
                            nc.vector.tensor_copy(
                                out=otv[0:n, vo + j * H_CHUNK:
                                        vo + (j + 1) * H_CHUNK],
                                in_=psv[g * C:g * C + n,
                                        bank * H_CHUNK:(bank + 1) * H_CHUNK])
                        else:
                            j = (g - GV) * NB + bank
                            nc.scalar.copy(
                                out=ots[0:n, so + j * H_CHUNK:
                                        so + (j + 1) * H_CHUNK],
                                in_=pss[g * C:g * C + n,
                                        bank * H_CHUNK:(bank + 1) * H_CHUNK])
                if b % 2 == 1 or b == EPC - 1:
                    # ALL stores ride the sync HWDGE queue: it is idle after
                    # the upfront slab triggers (wbufs == SLABS on the fp8
                    # path), so a compute-gated store at its head blocks
                    # nothing. A DMA trigger costs ~0.8us of ENGINE time on
                    # whichever engine issues it — on scalar that stole
                    # copy throughput (measured +5us tail); sync has
                    # nothing else to do. gpsimd SWDGE is NOT usable: its
                    # ucode spends ~2.2us per store generating small
                    # packets (measured +15us).
                    p0 = b - (b % 2)
                    ne = b - p0 + 1
                    # rows padded to a multiple of 16: HWDGE fans a DMA's
                    # descriptor lines over the 16 data engines starting at
                    # the first engine each trigger, so off-multiple line
                    # counts pile the remainder onto E64/E65 (measured:
                    # E64 carried 5.5x the average y bytes and became the
                    # critical engine, +13us). 16/32-row stores wrap all
                    # 16 engines exactly; the padding costs ~0.4 MiB.
                    rows = min(C, -(-max(pair_n, n) // 16) * 16)
                    yv = y[p0:p0 + ne, 0:rows, 0:vsplit]
                    nc.sync.dma_start(
                        out=yv.rearrange("e r h -> r e h"),
                        in_=otv[0:rows, 0:ne * vsplit])
                    if ots is not None:
                        ys = y[p0:p0 + ne, 0:rows, vsplit:H]
                        nc.sync.dma_start(
                            out=ys.rearrange("e r h -> r e h"),
                            in_=ots[0:rows, 0:ne * ssplit])
    nc.compile()
    return nc


def _get_nc(C: int, dt_name: str, ns: tuple | None = None):
    key = (C, dt_name, ns)
    if key not in _cache:
        _cache[key] = _build(C, dt_name, ns)
    return _cache[key]


def _np_dt(name):
    import ml_dtypes
    return {
        "float8e3": ml_dtypes.float8_e3m4,
        "bfloat16": ml_dtypes.bfloat16,
        "float32": np.float32,
        "float32r": np.float32,
    }[name]


def _prepare(x, w, chosen_experts, expert_weight, dt_name):
    """Host-side routing. Returns (C, ns, in_maps, row_lists) where
    row_lists[c][s] is the array of global row ids for core c, expert slot
    s, and ns[s] the per-slot valid row count baked into the kernel."""
    w_dtn, x_dtn, _ = DT_CONFIGS[dt_name]
    x = np.asarray(x, dtype=np.float32)
    w = np.asarray(w, dtype=np.float32)
    ce = np.asarray(chosen_experts).astype(np.int64).reshape(-1)      # [T*K]
    gw = np.asarray(expert_weight, dtype=np.float32).reshape(-1)      # [T*K]

    counts = np.bincount(ce, minlength=E)
    C = max(32, int(np.ceil(counts.max() / 32.0) * 32))

    order = np.argsort(ce, kind="stable")
    starts = np.zeros(E + 1, dtype=np.int64)
    np.cumsum(counts, out=starts[1:])

    xs = x * gw[:, None]  # fold router gate into rows (fp32)

    if w_dtn == "float8e3":
        # per-expert scale into the e3m4 range; inverse folded into x rows
        s = E3M4_SCALE_TARGET / np.maximum(
            np.abs(w).max(axis=(1, 2)), 1e-30)                        # [E]
    else:
        s = np.ones(E, dtype=np.float32)

    # Assign experts to (core, slot) in count-sorted rank groups: slot b on
    # every core gets an expert of rank group b, so one per-slot row count
    # (the group max) is tight for the whole SPMD program, y stores move
    # only real rows, and per-core load balances.
    rank = np.argsort(-counts, kind="stable")          # expert ids, big first
    assign = rank.reshape(EPC, N_CORES)                # [slot, core]
    ns = tuple(int(counts[assign[b]].max()) for b in range(EPC))

    EPS = _eps(w_dtn)
    in_maps, row_lists = [], []
    for c in range(N_CORES):
        xg = np.zeros((EPC * C, I_DIM), dtype=np.float32)
        rows_c = []
        for sl in range(EPC):
            e = int(assign[sl, c])
            rows = order[starts[e]:starts[e + 1]]
            xg[sl * C: sl * C + len(rows)] = xs[rows] * (1.0 / s[e])
            rows_c.append(rows)
        # [b, i*128+p, h] -> [k, p, e*ICH*H + i*H + h] (b = k*EPS+e):
        # contiguous per-partition slab lines, EPS experts per DMA slab
        eids = assign[:, c]
        wcore = (
            (w[eids] * s[eids, None, None])
            .reshape(EPC // EPS, EPS, I_CHUNKS, P, H)
            .transpose(0, 3, 1, 2, 4)
            .reshape(EPC // EPS, P, EPS * I_CHUNKS * H)
        )
        # [c, i*128+p] -> [p, i*EC + c]: one resident stationary tile
        xre = (
            xg.reshape(EPC * C, I_CHUNKS, P)
            .transpose(2, 1, 0)
            .reshape(P, I_CHUNKS * EPC * C)
        )
        in_maps.append({
            "wc": np.ascontiguousarray(wcore).astype(_np_dt(w_dtn)),
            "xT": np.ascontiguousarray(xre).astype(_np_dt(x_dtn)),
        })
        row_lists.append(rows_c)
    return C, ns, in_maps, row_lists


def _combine(results, row_lists, C, dt_name):
    # device stores row-major [n, H] per expert slot (compacted valid rows)
    # with H blocks permuted: vector-copied h-chunks first, then scalar's
    G = max(1, P // C)
    if H_CHUNKS % G != 0 or DT_CONFIGS[dt_name][0] not in ("bfloat16", "float8e3"):
        G = 1
    NB = H_CHUNKS // G
    NCOP = G * NB
    GV = max(1, G // 2)
    NV = GV * NB if G > 1 else G * NB
    blocks = [0] * NCOP  # blocks[j] = h-chunk stored in device col block j
    for g in range(G):
        for bank in range(NB):
            if G == 1 or g < GV:
                j = g * NB + bank
            else:
                j = NV + (g - GV) * NB + bank
            blocks[j] = bank * G + g
    yfull = np.empty((T * K_TOP, H), dtype=np.float32)
    for c in range(N_CORES):
        yc = np.asarray(results[c]["y"], dtype=np.float32)  # [EPC, C, H]
        yb = yc.reshape(EPC, C, NCOP, H_CHUNK)
        nat = np.empty_like(yb)
        nat[:, :, blocks, :] = yb
        nat = nat.reshape(EPC, C, H)
        for s, rows in enumerate(row_lists[c]):
            if len(rows):
                yfull[rows] = nat[s, : len(rows)]
    return yfull[0::2] + yfull[1::2]


def run(x, w, chosen_experts, expert_weight, dt_name=DEFAULT_DTYPE, **spmd_kwargs):
    from concourse.bass_utils import run_bass_kernel_spmd

    C, ns, in_maps, row_lists = _prepare(x, w, chosen_experts, expert_weight, dt_name)
    nc = _get_nc(C, dt_name, ns)
    res = run_bass_kernel_spmd(nc, in_maps, core_ids=list(range(N_CORES)), **spmd_kwargs)
    out = _combine(res.results, row_lists, C, dt_name)
    return out, res


def kernel(x, w, chosen_experts, expert_weight):
    out, _ = run(x, w, chosen_experts, expert_weight)
    return out



# revision 38
# speedup vs baseline: 1.1407x; 1.0191x over previous
"""MoE down-projection (grouped GEMM + topk combine) on 8 Trainium2 cores.

Strategy: expert-parallel. Each of the 8 cores owns E/8 = 16 experts and
receives (a) its experts' weight slabs and (b) the x rows routed to those
experts, gathered+gate-scaled+transposed on host, padded per expert to a
fixed capacity C. The device kernel is a block-diagonal grouped GEMM.
Weights stream through the PE as the moving operand (full rate); the few
x rows per expert are the stationary operand. G = 128//C h-chunks of one
expert run concurrently in separate PE column groups (tile_position),
each owning a contiguous C-partition range of a [128, H/G] PSUM tile.
The psum->sbuf copies then COMPACT the G groups' valid rows into the
free dim, producing a row-major [n, H] tile per expert so a single small
store moves only the real token rows. Host scatter-adds the rows back
into the [T, H] output.

The kernel is HBM-bandwidth bound on the weight stream (16 MiB/core fp8
at the ~430 GB/s = 16 engines x 27 GB/s per-core DMA ceiling), so the
default config stores w as fp8 E3M4 (per-expert scale folded into the x
rows) and the y output as compacted bf16 rows — total ~18.9 MiB/core
moved vs 512 MiB f32 for the naive form, at ~1.3e-2 relative error
(gate is 2e-2). Experts are assigned to (core, slot) in count-sorted
rank groups so one per-slot row count is tight for the whole SPMD
program and per-core load balances.

Hardware behaviors this kernel is shaped around (all measured here):
- every HWDGE DMA trigger costs ~0.6-0.9us of issuing-engine queue
  time; gpsimd SWDGE costs ~2.2us of ucode per store — so DMA COUNT is
  a first-class budget (w: 11 tapered slab pieces; y: 16 pair stores).
- the sync+scalar HWDGE queues share 8 HW completion-sem lanes; DMAs
  past 8-in-flight wait for lane recycling AT TRIGGER TIME.
- a DMA's descriptor lines fan over the 16 data engines restarting at
  the first engine each trigger: line counts that are multiples of 16
  (rows padded to 16/32) keep per-engine bytes exactly even; ragged
  stores piled 5.5x average on E64 and stretched the whole stream.
- any object (psum tile, sbuf tile, store) consumed by BOTH copy
  engines makes the tile framework fold multi-waits into a transitive
  vector->scalar wait chain (1-wait-per-instruction HW rule) that
  serializes all psum->sbuf copies; the vector and scalar pipelines
  here are fully object-disjoint (own psum tile, own ot tile, own
  store) and the host reorders the h-chunk halves.
- weight-release granularity tapers (2-expert pieces, then per-expert,
  then half-expert) so the copy engines (~1.4us/expert each) never
  inherit a multi-expert burst at the contended stream end.

Hardcoded problem shape (from the problem spec):
  x: [2048, 512] f32, w: [128, 512, 2048] f32,
  chosen_experts: [1024, 2] int, expert_weight: [1024, 2] f32 -> out [1024, 2048] f32
"""

import numpy as np

T = 1024
K_TOP = 2
E = 128
I_DIM = 512
H = 2048
N_CORES = 8
EPC = E // N_CORES  # experts per core = 16
P = 128             # partitions
I_CHUNKS = I_DIM // P       # 4
H_CHUNK = 512               # matmul moving free dim (fp32 PSUM bank)
H_CHUNKS = H // H_CHUNK     # 4

# matmul dtype config: name -> (w dtype, x dtype, y dtype)
#   float8e3  : w E3M4 (per-expert scaled), x bf16, y bf16 — half DMA traffic
#   float8e3x : both operands E3M4 (if mixed-dtype matmul is unsupported)
#   bfloat16  : both bf16, y f32
#   float32 / float32r: exact / reduced-precision f32
DT_CONFIGS = {
    "float8e3": ("float8e3", "bfloat16", "bfloat16"),
    "float8e3x": ("float8e3", "float8e3", "bfloat16"),
    "bfloat16": ("bfloat16", "bfloat16", "float32"),
    "float32": ("float32", "float32", "float32"),
    "float32r": ("float32r", "float32r", "float32"),
}
DEFAULT_DTYPE = "float8e3"
E3M4_SCALE_TARGET = 14.0  # keep clear of the 15.5 e3m4 max normal

_cache = {}


def _w_bytes(w_dtn):
    return 1 if w_dtn == "float8e3" else (2 if w_dtn == "bfloat16" else 4)


def _eps(w_dtn):
    """Experts per weight-slab BUFFER (4 MiB fp8 -> 4 slabs resident =
    the whole 16 MiB working set). DMA granularity within a slab is
    finer (see issue_slab): the piece count is kept near the 8 HW
    completion-sem lanes shared by the sync+scalar DGE queues — DMAs
    past 8-in-flight wait for lane recycling at trigger time, which can
    serialize the queue behind data completions (measured +6us when
    splits pushed the count to 15)."""
    mib = 4 if _w_bytes(w_dtn) == 1 else 2
    return max(1, mib * 1024 * 1024 // (P * I_CHUNKS * H * _w_bytes(w_dtn)))


def _build(C: int, dt_name: str, ns: tuple | None = None):
    """ns: per-slot valid row counts (same for every core by construction —
    the host assigns experts to slots in count-sorted rank groups). When
    given, y stores move only those rows."""
    import concourse.mybir as mybir
    import concourse.tile as tile
    from concourse import bacc

    w_dtn, x_dtn, y_dtn = DT_CONFIGS[dt_name]
    w_dt = getattr(mybir.dt, w_dtn)
    x_dt = getattr(mybir.dt, x_dtn)
    y_dt = getattr(mybir.dt, y_dtn)
    w_bytes = _w_bytes(w_dtn)
    EPS = _eps(w_dtn)
    SLABS = EPC // EPS
    SLAB_COLS = EPS * I_CHUNKS * H
    # G = 128//C PE column groups run one expert's G h-chunks concurrently;
    # expert b's H chunk h goes to psum partitions (h%G)*C..+C, bank cols
    # (h//G)*512..+512. fp32 rejects tile_position col-tiling.
    G = max(1, P // C)
    if H_CHUNKS % G != 0 or w_dtn not in ("bfloat16", "float8e3"):
        G = 1
    NB = H_CHUNKS // G
    PPART = G * C
    # keep the whole weight working set resident when it fits (fp8: 16 MiB)
    wbufs = SLABS if w_bytes == 1 else (6 if w_bytes == 2 else 3)
    # ot tiles are small ([C, H/2] y_dt); deep rotation keeps the
    # cast->store WAR chain from ever pacing the compute pipeline
    obufs = 8 if w_bytes <= 2 else 2
    # two psum tiles per expert (one per copy engine), each NB banks deep;
    # size the rotation to the 8 PSUM banks (G=4: 2x4x1, G=2: 2x2x2).
    # G == 1 uses one tile of NB banks per expert.
    pbufs = max(1, 4 // NB) if G > 1 else max(1, 8 // (2 * NB))

    nc = bacc.Bacc()
    # wc host-prearranged: [k, p, e*ICH*H + i*H + h] = w[k*EPS+e, i*128+p, h]
    # so each partition's slab line is 1 contiguous run per DMA
    wc = nc.declare_dram_parameter("wc", [SLABS, P, SLAB_COLS], w_dt, isOutput=False)
    # x host-prearranged: [p, i*EC + c] = x[i*128+p, c] (EC = EPC*C) so the
    # whole stationary operand arrives in ONE small DMA before the w flood
    xT = nc.declare_dram_parameter("xT", [P, I_CHUNKS * EPC * C], x_dt, isOutput=False)
    # y rows are stored compacted: y[b, r] = full H row for valid row r < ns[b]
    y = nc.declare_dram_parameter("y", [EPC, C, H], y_dt, isOutput=True)

    with tile.TileContext(nc) as tc:
        with (
            tc.tile_pool(name="wp", bufs=wbufs) as wp,
            tc.tile_pool(name="xp", bufs=1) as xp,
            tc.tile_pool(name="ppv", bufs=pbufs, space="PSUM") as ppv,
            tc.tile_pool(name="pps", bufs=pbufs, space="PSUM") as pps,
            tc.tile_pool(name="opv", bufs=obufs) as opv,
            tc.tile_pool(name="ops", bufs=obufs) as ops,
        ):
            # x rows (stationary operands) go out on the scalar HWDGE queue:
            # the sync ring then issues w slab triggers back-to-back from the
            # first kernel instruction, starting the weight stream ~0.9us
            # earlier. x interleaves with slab 0 on the shared engines and
            # still lands long before the first matmul needs it. (x is not
            # compute-gated, so it cannot block anything through scalar's
            # sem lanes the way compute-gated y stores would.)
            EC = EPC * C
            xt_all = xp.tile([P, I_CHUNKS * EC], x_dt, tag="x", name="x")
            nc.scalar.dma_start(out=xt_all[:], in_=xT[:])
            xtiles = [xt_all[:, i * EC:(i + 1) * EC] for i in range(I_CHUNKS)]

            def issue_slab(k):
                # Weight-release granularity tapers toward the stream end:
                # early slabs whole (maximum trigger slack — the first 8
                # HWDGE DMAs hold the 8 completion-sem lanes and issue
                # upfront), then 2-expert halves, then per-expert, and the
                # very last expert in two i-chunk halves. Coarse releases
                # bunch experts into the copy engines (1.38us/expert drain
                # vs 2.66us/expert stream pace is fine steady-state, but a
                # 4-expert burst at the contended stream end added ~5us of
                # tail); the taper keeps the tail per-expert while w DMA
                # count stays at 11 (+x = 12; the few recycle-waits land on
                # long-completed lanes and never bind).
                wt = wp.tile([P, SLAB_COLS], w_dt, tag="w0",
                             name=f"w{k}", bufs=wbufs)
                ecols = I_CHUNKS * H
                if k == SLABS - 1 and EPS > 1:
                    for e in range(EPS - 1):
                        nc.sync.dma_start(out=wt[:, e * ecols:(e + 1) * ecols],
                                          in_=wc[k, :, e * ecols:(e + 1) * ecols])
                    lo = (EPS - 1) * ecols
                    half = I_CHUNKS // 2 * H
                    for h2 in range(2):
                        nc.sync.dma_start(
                            out=wt[:, lo + h2 * half:lo + (h2 + 1) * half],
                            in_=wc[k, :, lo + h2 * half:lo + (h2 + 1) * half])
                elif EPS >= 4:
                    # 2-expert release pieces: a whole 4-expert slab dumps
                    # ~5.5us of copy work on each copy engine at once while
                    # the stream feeds ~1us/expert of slack — the backlog
                    # compounds under contention and drains as tail
                    hcols = (EPS // 2) * ecols
                    for h2 in range(2):
                        nc.sync.dma_start(
                            out=wt[:, h2 * hcols:(h2 + 1) * hcols],
                            in_=wc[k, :, h2 * hcols:(h2 + 1) * hcols])
                else:
                    nc.sync.dma_start(out=wt[:], in_=wc[k])
                return wt

            # issue every slab DMA upfront when all buffers are resident
            # (fp8: 4 x 4 MiB); otherwise stream with buffer rotation
            wts = {k: issue_slab(k) for k in range(min(wbufs, SLABS))}

            # Copy-engine split: PE column groups g < GV accumulate in the
            # "vector" psum tile (partitions [0, GV*C)), groups g >= GV in
            # the "scalar" psum tile (partitions [GV*C, PPART)). Vector
            # copies compact the first half's h-chunks into otv, scalar the
            # second half's into ots, and each half has its OWN store.
            # CRITICAL: the two engines get fully DISJOINT object graphs —
            # separate psum tiles, separate ot tiles, separate stores — so
            # NO instruction ever depends on both engines' progress. Any
            # shared object (one psum tile, one ot tile, or one combined
            # store) makes the tile framework fold the resulting multi-wait
            # into a transitive V->S->V->S chain (single-wait-per-
            # instruction HW rule), serializing ALL copies at 2.76us/expert
            # — slower than the 2.66us/expert weight stream (measured
            # repeatedly as an ~10us tail). Disjoint halves run at
            # 1.38us/expert per engine. Host unpacks the h-chunk order
            # (vector's h-chunks first, then scalar's).
            GV = max(1, G // 2)  # column groups handled by vector
            NV = GV * NB if G > 1 else G * NB
            vsplit = NV * H_CHUNK
            ssplit = H - vsplit

            for b in range(EPC):
                k = b // EPS
                if k not in wts:
                    wts[k] = issue_slab(k)
                wt = wts[k]
                wo = (b % EPS) * I_CHUNKS * H
                psv = ppv.tile([PPART, NB * H_CHUNK], mybir.dt.float32,
                               tag="psv", name=f"psv{b}")
                pss = (pps.tile([PPART, NB * H_CHUNK], mybir.dt.float32,
                                tag="pss", name=f"pss{b}") if G > 1 else None)
                for i in range(I_CHUNKS):
                    for h in range(H_CHUNKS):
                        g, bank = h % G, h // G
                        ps = psv if (G == 1 or g < GV) else pss
                        nc.tensor.matmul(
                            ps[g * C:(g + 1) * C,
                               bank * H_CHUNK:(bank + 1) * H_CHUNK],
                            lhsT=xtiles[i][:, b * C:(b + 1) * C],
                            rhs=wt[:, wo + i * H + h * H_CHUNK:
                                   wo + i * H + (h + 1) * H_CHUNK],
                            start=(i == 0),
                            stop=(i == I_CHUNKS - 1),
                            tile_position=(0, g * C) if G > 1 else None,
                        )
                # Compact each half's valid rows into the FREE dim during
                # the psum->sbuf cast: (g, bank) block -> [n, 512] col
                # block, giving row-major [n, H/2] tiles. Partition bases
                # stay 32-aligned (engine ops reject unaligned bases) and
                # y stores then move only real token rows — y bytes drop
                # ~2x off the shared DMA engines that carry the w stream.
                #
                # Stores cover an EXPERT PAIR (rows padded to the pair max;
                # slots are count-sorted so the overhead is ~6%): 16 HWDGE
                # DMAs instead of 32. Every HWDGE DMA costs ~0.7us of queue
                # time, recycles one of only 8 HW completion-sem lanes
                # (a 33rd+ DMA serializes the tail at ~0.94us/store,
                # measured as a 10us dribble), and small-line DMAs fan
                # poorly over the 16 data engines (lines restart at E64
                # each trigger).
                n = C if ns is None else max(1, min(int(ns[b]), C))
                if b % 2 == 0:
                    otv = opv.tile([C, 2 * vsplit], y_dt, tag="ov",
                                   name=f"ov{b}")
                    ots = (ops.tile([C, 2 * ssplit], y_dt, tag="os",
                                    name=f"os{b}") if ssplit else None)
                    pair_n = n
                vo = (b % 2) * vsplit
                so = (b % 2) * ssplit
                for g in range(G):
                    for bank in range(NB):
                        if G == 1 or g < GV:
                            j = g * NB + bank
                            nc.vector.tensor_copy(
                                out=otv[0:n, vo + j * H_CHUNK:
                                        vo + (j + 1) * H_CHUNK],
                                in_=psv[g * C:g * C + n,
                                        bank * H_CHUNK:(bank + 1) * H_CHUNK])
                        else:
                            j = (g - GV) * NB + bank
                            nc.scalar.copy(
                                out=ots[0:n, so + j * H_CHUNK:
                                        so + (j + 1) * H_CHUNK],
                                in_=pss[g * C:g * C + n,
                                        bank * H_CHUNK:(bank + 1) * H_CHUNK])
                if b % 2 == 1 or b == EPC - 1:
                    # ALL stores ride the sync HWDGE queue: it is idle after
                    # the upfront slab triggers (wbufs == SLABS on the fp8
                    # path), so a compute-gated store at its head blocks
                    # nothing. A DMA trigger costs ~0.8us of ENGINE time on
                    # whichever engine issues it — on scalar that stole
                    # copy throughput (measured +5us tail); sync has
                    # nothing else to do. gpsimd SWDGE is NOT usable: its
                    # ucode spends ~2.2us per store generating small
                    # packets (measured +15us).
                    p0 = b - (b % 2)
                    ne = b - p0 + 1
                    # rows padded to a multiple of 16: HWDGE fans a DMA's
                    # descriptor lines over the 16 data engines starting at
                    # the first engine each trigger, so off-multiple line
                    # counts pile the remainder onto E64/E65 (measured:
                    # E64 carried 5.5x the average y bytes and became the
                    # critical engine, +13us). 16/32-row stores wrap all
                    # 16 engines exactly; the padding costs ~0.4 MiB.
                    rows = min(C, -(-max(pair_n, n) // 16) * 16)
                    yv = y[p0:p0 + ne, 0:rows, 0:vsplit]
                    nc.sync.dma_start(
                        out=yv.rearrange("e r h -> r e h"),
                        in_=otv[0:rows, 0:ne * vsplit])
                    if ots is not None:
                        ys = y[p0:p0 + ne, 0:rows, vsplit:H]
                        nc.sync.dma_start(
                            out=ys.rearrange("e r h -> r e h"),
                            in_=ots[0:rows, 0:ne * ssplit])
    nc.compile()
    return nc


def _get_nc(C: int, dt_name: str, ns: tuple | None = None):
    key = (C, dt_name, ns)
    if key not in _cache:
        _cache[key] = _build(C, dt_name, ns)
    return _cache[key]


def _np_dt(name):
    import ml_dtypes
    return {
        "float8e3": ml_dtypes.float8_e3m4,
        "bfloat16": ml_dtypes.bfloat16,
        "float32": np.float32,
        "float32r": np.float32,
    }[name]


def _prepare(x, w, chosen_experts, expert_weight, dt_name):
    """Host-side routing. Returns (C, ns, in_maps, row_lists) where
    row_lists[c][s] is the array of global row ids for core c, expert slot
    s, and ns[s] the per-slot valid row count baked into the kernel."""
    w_dtn, x_dtn, _ = DT_CONFIGS[dt_name]
    x = np.asarray(x, dtype=np.float32)
    w = np.asarray(w, dtype=np.float32)
    ce = np.asarray(chosen_experts).astype(np.int64).reshape(-1)      # [T*K]
    gw = np.asarray(expert_weight, dtype=np.float32).reshape(-1)      # [T*K]

    counts = np.bincount(ce, minlength=E)
    C = max(32, int(np.ceil(counts.max() / 32.0) * 32))

    order = np.argsort(ce, kind="stable")
    starts = np.zeros(E + 1, dtype=np.int64)
    np.cumsum(counts, out=starts[1:])

    xs = x * gw[:, None]  # fold router gate into rows (fp32)

    if w_dtn == "float8e3":
        # per-expert scale into the e3m4 range; inverse folded into x rows
        s = E3M4_SCALE_TARGET / np.maximum(
            np.abs(w).max(axis=(1, 2)), 1e-30)                        # [E]
    else:
        s = np.ones(E, dtype=np.float32)

    # Assign experts to (core, slot) in count-sorted rank groups: slot b on
    # every core gets an expert of rank group b, so one per-slot row count
    # (the group max) is tight for the whole SPMD program, y stores move
    # only real rows, and per-core load balances.
    rank = np.argsort(-counts, kind="stable")          # expert ids, big first
    assign = rank.reshape(EPC, N_CORES)                # [slot, core]
    ns = tuple(int(counts[assign[b]].max()) for b in range(EPC))

    EPS = _eps(w_dtn)
    in_maps, row_lists = [], []
    for c in range(N_CORES):
        xg = np.zeros((EPC * C, I_DIM), dtype=np.float32)
        rows_c = []
        for sl in range(EPC):
            e = int(assign[sl, c])
            rows = order[starts[e]:starts[e + 1]]
            xg[sl * C: sl * C + len(rows)] = xs[rows] * (1.0 / s[e])
            rows_c.append(rows)
        # [b, i*128+p, h] -> [k, p, e*ICH*H + i*H + h] (b = k*EPS+e):
        # contiguous per-partition slab lines, EPS experts per DMA slab
        eids = assign[:, c]
        wcore = (
            (w[eids] * s[eids, None, None])
            .reshape(EPC // EPS, EPS, I_CHUNKS, P, H)
            .transpose(0, 3, 1, 2, 4)
            .reshape(EPC // EPS, P, EPS * I_CHUNKS * H)
        )
        # [c, i*128+p] -> [p, i*EC + c]: one resident stationary tile
        xre = (
            xg.reshape(EPC * C, I_CHUNKS, P)
            .transpose(2, 1, 0)
            .reshape(P, I_CHUNKS * EPC * C)
        )
        in_maps.append({
            "wc": np.ascontiguousarray(wcore).astype(_np_dt(w_dtn)),
            "xT": np.ascontiguousarray(xre).astype(_np_dt(x_dtn)),
        })
        row_lists.append(rows_c)
    return C, ns, in_maps, row_lists


def _combine(results, row_lists, C, dt_name):
    # device stores row-major [n, H] per expert slot (compacted valid rows)
    # with H blocks permuted: vector-copied h-chunks first, then scalar's
    G = max(1, P // C)
    if H_CHUNKS % G != 0 or DT_CONFIGS[dt_name][0] not in ("bfloat16", "float8e3"):
        G = 1
    NB = H_CHUNKS // G
    NCOP = G * NB
    GV = max(1, G // 2)
    NV = GV * NB if G > 1 else G * NB
    blocks = [0] * NCOP  # blocks[j] = h-chunk stored in device col block j
    for g in range(G):
        for bank in range(NB):
            if G == 1 or g < GV:
                j = g * NB + bank
            else:
                j = NV + (g - GV) * NB + bank
            blocks[j] = bank * G + g
    yfull = np.empty((T * K_TOP, H), dtype=np.float32)
    for c in range(N_CORES):
        yc = np.asarray(results[c]["y"], dtype=np.float32)  # [EPC, C, H]
        yb = yc.reshape(EPC, C, NCOP, H_CHUNK)
        nat = np.empty_like(yb)
        nat[:, :, blocks, :] = yb
        nat = nat.reshape(EPC, C, H)
        for s, rows in enumerate(row_lists[c]):
            if len(rows):
                yfull[rows] = nat[s, : len(rows)]
    return yfull[0::2] + yfull[1::2]


def run(x, w, chosen_experts, expert_weight, dt_name=DEFAULT_DTYPE, **spmd_kwargs):
    from concourse.bass_utils import run_bass_kernel_spmd

    C, ns, in_maps, row_lists = _prepare(x, w, chosen_experts, expert_weight, dt_name)
    nc = _get_nc(C, dt_name, ns)
    res = run_bass_kernel_spmd(nc, in_maps, core_ids=list(range(N_CORES)), **spmd_kwargs)
    out = _combine(res.results, row_lists, C, dt_name)
    return out, res


def kernel(x, w, chosen_experts, expert_weight):
    out, _ = run(x, w, chosen_experts, expert_weight)
    return out

